# revision 5
# baseline (speedup 1.0000x reference)
"""Category-specific MLP (MoE-style routing) for Trainium2, 8 NeuronCores.

Reference computation (per token n):
    h   = relu(x[n] @ W1[cat[n]] + b1[cat[n]])      x:[N,128]  W1:[100,128,128]
    out = h @ W2[cat[n]] + b2[cat[n]]               W2:[100,128,64]

Strategy (expert-parallel, MoE-style):
  * Host: sort tokens by category. Split any category with more than 512
    tokens into work items of <=512 tokens. Sort items by size (desc) and
    assign item of rank r to (core r%8, slot r//8). All cores run the same
    SPMD program with S slots; slot s has a fixed column capacity
    caps[s] = size of the largest item assigned to that slot across cores,
    so the instruction stream and shapes are identical on every core while
    padding stays minimal (~5% for the target distribution).
  * Device (per core): everything kept feature-on-partitions (transposed).
    The per-core input is ONE [128, Z] f32 blob:
        [ b1 S | b2 S (rows 0:64) | group0: w1 | w2 | xT | group1: ... ]
    so each group of 4 slots is loaded with a single large DMA.  Per slot:
        psum1 = w1_s^T @ xT_s          (PE,  [128,B])
        h     = relu(psum1 + b1_s)     (ACT, PSUM->SBUF)
        psum2 = w2_s^T @ h             (PE,  [64,B])
        out_s = psum2 + b2_s           (ACT Identity, PSUM->SBUF)
    -> outT [64, T] stored per group. All PSUM consumption is on ScalarE so
    each fp32 matmul carries at most ONE sync wait (the walrus S3_LW
    ldweights struct only encodes a single wait command).
  * Host: scatter outT columns back to the original token order.
"""

from contextlib import ExitStack

import numpy as np

import concourse.bass as bass
import concourse.mybir as mybir
import concourse.tile as tile
from concourse import bacc
from concourse.bass_utils import run_bass_kernel_spmd

N, C, D, H, O = 8192, 100, 128, 128, 64
NCORES = 8
MAX_ITEM = 512      # PSUM bank / fp32 moving-operand limit
SLOTS_PER_GROUP = 4

F32 = mybir.dt.float32


def _plan(cat_ids: np.ndarray):
    """Host-side routing plan: work items -> (core, slot) assignment."""
    cat_ids = np.asarray(cat_ids).astype(np.int64)
    counts = np.bincount(cat_ids, minlength=C)
    order = np.argsort(cat_ids, kind="stable")          # token ids sorted by cat
    starts = np.zeros(C, dtype=np.int64)
    starts[1:] = np.cumsum(counts)[:-1]

    items = []                                          # (cat, start_in_cat, len)
    for c in range(C):
        cnt = int(counts[c])
        o = 0
        while o < cnt:
            ln = min(MAX_ITEM, cnt - o)
            items.append((c, o, ln))
            o += ln
    items.sort(key=lambda it: -it[2])

    S = (len(items) + NCORES - 1) // NCORES
    grid = [[None] * NCORES for _ in range(S)]          # grid[s][k] = item|None
    for r, it in enumerate(items):
        grid[r // NCORES][r % NCORES] = it
    caps = tuple(max(1, max((it[2] for it in row if it is not None), default=1))
                 for row in grid)
    offs = np.zeros(S + 1, dtype=np.int64)
    offs[1:] = np.cumsum(caps)
    T = int(offs[-1])

    groups = [(s0, min(s0 + SLOTS_PER_GROUP, S))
              for s0 in range(0, S, SLOTS_PER_GROUP)]
    # blob column layout: [b1 S | b2 S | per group: w1 ns*H | w2 ns*O | x cols]
    gpos = []
    pos = 2 * S
    for (s0, s1) in groups:
        ns = s1 - s0
        cols = int(offs[s1] - offs[s0])
        gpos.append(pos)
        pos += ns * (H + O) + cols
    Z = pos

    return {
        "order": order, "starts": starts, "grid": grid,
        "S": S, "caps": caps, "offs": offs, "T": T,
        "groups": groups, "gpos": gpos, "Z": Z,
    }


_NC_CACHE: dict = {}


def _build_nc(plan):
    S, caps, T, Z = plan["S"], plan["caps"], plan["T"], plan["Z"]
    key = (S, caps)
    if key in _NC_CACHE:
        return _NC_CACHE[key]

    offs, groups, gpos = plan["offs"], plan["groups"], plan["gpos"]

    nc = bacc.Bacc("TRN2", target_bir_lowering=False, debug=False)
    blob_d = nc.dram_tensor("blob", [128, Z], F32, kind="ExternalInput").ap()
    out_d = nc.dram_tensor("outT", [O, T], F32, kind="ExternalOutput").ap()

    with tile.TileContext(nc) as tc, ExitStack() as ctx:
        consts = ctx.enter_context(tc.tile_pool(name="consts", bufs=1))
        loads = ctx.enter_context(tc.tile_pool(name="loads", bufs=2))
        hbuf = ctx.enter_context(tc.tile_pool(name="hbuf", bufs=3))
        obuf = ctx.enter_context(tc.tile_pool(name="obuf", bufs=2))
        ps1p = ctx.enter_context(tc.tile_pool(name="ps1p", bufs=3, space="PSUM"))
        ps2p = ctx.enter_context(tc.tile_pool(name="ps2p", bufs=3, space="PSUM"))

        bias = consts.tile([128, 2 * S], F32)
        nc.sync.dma_start(out=bias, in_=blob_d[:, 0:2 * S])

        for gi, (s0, s1) in enumerate(groups):
            ns = s1 - s0
            co0, co1 = int(offs[s0]), int(offs[s1])
            cols = co1 - co0
            W_g = ns * (H + O) + cols
            g_sb = loads.tile([128, W_g], F32, tag="blob")
            nc.sync.dma_start(out=g_sb, in_=blob_d[:, gpos[gi]:gpos[gi] + W_g])
            w1v = g_sb[:, 0:ns * H]
            w2v = g_sb[:, ns * H:ns * (H + O)]
            xv = g_sb[:, ns * (H + O):W_g]
            o_g = obuf.tile([O, cols], F32, tag="o")
            for s in range(s0, s1):
                i = s - s0
                B = int(caps[s])
                lo = int(offs[s]) - co0
                ps1 = ps1p.tile([H, B], F32, tag="ps1")
                nc.tensor.matmul(ps1, lhsT=w1v[:, i * H:(i + 1) * H],
                                 rhs=xv[:, lo:lo + B], start=True, stop=True)
                h_sb = hbuf.tile([H, B], F32, tag="h")
                nc.scalar.activation(h_sb, ps1,
                                     mybir.ActivationFunctionType.Relu,
                                     bias=bias[:, s:s + 1])
                ps2 = ps2p.tile([O, B], F32, tag="ps2")
                nc.tensor.matmul(ps2, lhsT=w2v[:, i * O:(i + 1) * O],
                                 rhs=h_sb, start=True, stop=True)
                nc.scalar.activation(o_g[:, lo:lo + B], ps2,
                                     mybir.ActivationFunctionType.Identity,
                                     bias=bias[0:O, S + s:S + s + 1])
            nc.sync.dma_start(out=out_d[:, co0:co1], in_=o_g)

    nc.compile()
    _NC_CACHE[key] = nc
    return nc


def _shard_inputs(x, W1, b1, W2, b2, plan):
    S, offs, Z = plan["S"], plan["offs"], plan["Z"]
    order, starts, grid = plan["order"], plan["starts"], plan["grid"]
    groups, gpos = plan["groups"], plan["gpos"]

    in_maps = []
    for k in range(NCORES):
        blob = np.zeros((128, Z), dtype=np.float32)
        for gi, (s0, s1) in enumerate(groups):
            ns = s1 - s0
            co0 = int(offs[s0])
            p = gpos[gi]
            for s in range(s0, s1):
                it = grid[s][k]
                if it is None:
                    continue
                i = s - s0
                c, o, ln = it
                toks = order[starts[c] + o: starts[c] + o + ln]
                blob[:, p + i * H:p + (i + 1) * H] = W1[c]
                blob[:, p + ns * H + i * O:p + ns * H + (i + 1) * O] = W2[c]
                xoff = p + ns * (H + O) + (int(offs[s]) - co0)
                blob[:, xoff:xoff + ln] = x[toks].T
                blob[:, s] = b1[c]
                blob[0:O, S + s] = b2[c]
        in_maps.append({"blob": blob})
    return in_maps


def _unshard(results, plan):
    S, offs = plan["S"], plan["offs"]
    order, starts, grid = plan["order"], plan["starts"], plan["grid"]
    out = np.empty((N, O), dtype=np.float32)
    for k in range(NCORES):
        outT = results[k]["outT"]
        for s in range(S):
            it = grid[s][k]
            if it is None:
                continue
            c, o, ln = it
            toks = order[starts[c] + o: starts[c] + o + ln]
            off = int(offs[s])
            out[toks] = outT[:, off:off + ln].T
    return out


def _execute(x, cat_ids, W1, b1, W2, b2, trace=False):
    x = np.asarray(x, dtype=np.float32)
    W1 = np.asarray(W1, dtype=np.float32)
    b1 = np.asarray(b1, dtype=np.float32)
    W2 = np.asarray(W2, dtype=np.float32)
    b2 = np.asarray(b2, dtype=np.float32)

    plan = _plan(cat_ids)
    nc = _build_nc(plan)
    in_maps = _shard_inputs(x, W1, b1, W2, b2, plan)
    res = run_bass_kernel_spmd(nc, in_maps, list(range(NCORES)), trace=trace)
    out = _unshard(res.results, plan)
    return out, res


def kernel(x, cat_ids, W1, b1, W2, b2):
    out, _ = _execute(x, cat_ids, W1, b1, W2, b2, trace=False)
    return out


# revision 6
# speedup vs baseline: 1.1013x; 1.1013x over previous
"""Category-specific MLP (MoE-style routing) for Trainium2, 8 NeuronCores.

Reference computation (per token n):
    h   = relu(x[n] @ W1[cat[n]] + b1[cat[n]])      x:[N,128]  W1:[100,128,128]
    out = h @ W2[cat[n]] + b2[cat[n]]               W2:[100,128,64]

Strategy (expert-parallel, MoE-style):
  * Host: sort tokens by category. Split any category with more than 512
    tokens into work items of <=512 tokens. Sort items by size (desc) and
    assign item of rank r to (core r%8, slot r//8). All cores run the same
    SPMD program with S slots; slot s has a fixed column capacity
    caps[s] = size of the largest item assigned to that slot across cores,
    so the instruction stream and shapes are identical on every core while
    padding stays minimal (~5% for the target distribution).
  * Device (per core): everything kept feature-on-partitions (transposed).
    The per-core input is ONE [128, Z] f32 blob:
        [ b1 S | b2 S (rows 0:64) | group0: w1 | w2 | xT | group1: ... ]
    so each group of 4 slots is loaded with a single large DMA.  Per slot:
        psum1 = w1_s^T @ xT_s          (PE,  [128,B])
        h     = relu(psum1 + b1_s)     (ACT, PSUM->SBUF)
        psum2 = w2_s^T @ h             (PE,  [64,B])
        out_s = psum2 + b2_s           (ACT Identity, PSUM->SBUF)
    -> outT [64, T] stored per group. All PSUM consumption is on ScalarE so
    each fp32 matmul carries at most ONE sync wait (the walrus S3_LW
    ldweights struct only encodes a single wait command).
  * Host: scatter outT columns back to the original token order.
"""

from contextlib import ExitStack

import numpy as np

import concourse.bass as bass
import concourse.mybir as mybir
import concourse.tile as tile
from concourse import bacc
from concourse.bass_utils import run_bass_kernel_spmd

N, C, D, H, O = 8192, 100, 128, 128, 64
NCORES = 8
MAX_ITEM = 512      # PSUM bank / fp32 moving-operand limit
SLOTS_PER_GROUP = 4

F32 = mybir.dt.float32


def _plan(cat_ids: np.ndarray):
    """Host-side routing plan: work items -> (core, slot) assignment."""
    cat_ids = np.asarray(cat_ids).astype(np.int64)
    counts = np.bincount(cat_ids, minlength=C)
    order = np.argsort(cat_ids, kind="stable")          # token ids sorted by cat
    starts = np.zeros(C, dtype=np.int64)
    starts[1:] = np.cumsum(counts)[:-1]

    items = []                                          # (cat, start_in_cat, len)
    for c in range(C):
        cnt = int(counts[c])
        o = 0
        while o < cnt:
            ln = min(MAX_ITEM, cnt - o)
            items.append((c, o, ln))
            o += ln
    items.sort(key=lambda it: -it[2])

    S = (len(items) + NCORES - 1) // NCORES
    grid = [[None] * NCORES for _ in range(S)]          # grid[s][k] = item|None
    for r, it in enumerate(items):
        grid[r // NCORES][r % NCORES] = it
    caps = tuple(max(1, max((it[2] for it in row if it is not None), default=1))
                 for row in grid)
    offs = np.zeros(S + 1, dtype=np.int64)
    offs[1:] = np.cumsum(caps)
    T = int(offs[-1])

    groups = [(s0, min(s0 + SLOTS_PER_GROUP, S))
              for s0 in range(0, S, SLOTS_PER_GROUP)]
    # blob column layout: [b1 S | b2 S | per group: w1 ns*H | w2 ns*O | x cols]
    gpos = []
    pos = 2 * S
    for (s0, s1) in groups:
        ns = s1 - s0
        cols = int(offs[s1] - offs[s0])
        gpos.append(pos)
        pos += ns * (H + O) + cols
    Z = pos

    return {
        "order": order, "starts": starts, "grid": grid,
        "S": S, "caps": caps, "offs": offs, "T": T,
        "groups": groups, "gpos": gpos, "Z": Z,
    }


_NC_CACHE: dict = {}


def _build_nc(plan):
    S, caps, T, Z = plan["S"], plan["caps"], plan["T"], plan["Z"]
    key = (S, caps)
    if key in _NC_CACHE:
        return _NC_CACHE[key]

    offs, groups, gpos = plan["offs"], plan["groups"], plan["gpos"]

    nc = bacc.Bacc("TRN2", target_bir_lowering=False, debug=False)
    blob_d = nc.dram_tensor("blob", [128, Z], F32, kind="ExternalInput").ap()
    out_d = nc.dram_tensor("outT", [O, T], F32, kind="ExternalOutput").ap()

    with tile.TileContext(nc) as tc, ExitStack() as ctx:
        consts = ctx.enter_context(tc.tile_pool(name="consts", bufs=1))
        loads = ctx.enter_context(tc.tile_pool(name="loads", bufs=2))
        hbuf = ctx.enter_context(tc.tile_pool(name="hbuf", bufs=8))
        obuf = ctx.enter_context(tc.tile_pool(name="obuf", bufs=2))
        ps1p = ctx.enter_context(tc.tile_pool(name="ps1p", bufs=4, space="PSUM"))
        ps2p = ctx.enter_context(tc.tile_pool(name="ps2p", bufs=4, space="PSUM"))

        bias = consts.tile([128, 2 * S], F32)
        nc.sync.dma_start(out=bias, in_=blob_d[:, 0:2 * S])

        for gi, (s0, s1) in enumerate(groups):
            ns = s1 - s0
            co0, co1 = int(offs[s0]), int(offs[s1])
            cols = co1 - co0
            W_g = ns * (H + O) + cols
            g_sb = loads.tile([128, W_g], F32, tag="blob")
            nc.sync.dma_start(out=g_sb, in_=blob_d[:, gpos[gi]:gpos[gi] + W_g])
            w1v = g_sb[:, 0:ns * H]
            w2v = g_sb[:, ns * H:ns * (H + O)]
            xv = g_sb[:, ns * (H + O):W_g]
            o_g = obuf.tile([O, cols], F32, tag="o")
            # Two-phase PE order: all layer-1 matmuls of the group
            # back-to-back, then all layer-2 matmuls; PSUM evacuation
            # (bias+relu / bias-add) rides on DVE so each matmul carries a
            # single fresh semaphore wait and PE never ping-pongs.
            ps1s, hs, ps2s = [], [], []
            for s in range(s0, s1):
                i, B = s - s0, int(caps[s])
                lo = int(offs[s]) - co0
                ps1 = ps1p.tile([H, B], F32, tag="ps1", name=f"ps1_{s}")
                nc.tensor.matmul(ps1, lhsT=w1v[:, i * H:(i + 1) * H],
                                 rhs=xv[:, lo:lo + B], start=True, stop=True)
                ps1s.append(ps1)
            for s in range(s0, s1):
                i, B = s - s0, int(caps[s])
                h_sb = hbuf.tile([H, B], F32, tag="h", name=f"h_{s}")
                nc.vector.tensor_scalar(h_sb, ps1s[i], bias[:, s:s + 1], 0.0,
                                        mybir.AluOpType.add, mybir.AluOpType.max)
                hs.append(h_sb)
            for s in range(s0, s1):
                i, B = s - s0, int(caps[s])
                ps2 = ps2p.tile([O, B], F32, tag="ps2", name=f"ps2_{s}")
                nc.tensor.matmul(ps2, lhsT=w2v[:, i * O:(i + 1) * O],
                                 rhs=hs[i], start=True, stop=True)
                ps2s.append(ps2)
            for s in range(s0, s1):
                i, B = s - s0, int(caps[s])
                lo = int(offs[s]) - co0
                nc.vector.tensor_scalar_add(o_g[:, lo:lo + B], ps2s[i],
                                            bias[0:O, S + s:S + s + 1])
            nc.sync.dma_start(out=out_d[:, co0:co1], in_=o_g)

    nc.compile()
    _NC_CACHE[key] = nc
    return nc


def _shard_inputs(x, W1, b1, W2, b2, plan):
    S, offs, Z = plan["S"], plan["offs"], plan["Z"]
    order, starts, grid = plan["order"], plan["starts"], plan["grid"]
    groups, gpos = plan["groups"], plan["gpos"]

    in_maps = []
    for k in range(NCORES):
        blob = np.zeros((128, Z), dtype=np.float32)
        for gi, (s0, s1) in enumerate(groups):
            ns = s1 - s0
            co0 = int(offs[s0])
            p = gpos[gi]
            for s in range(s0, s1):
                it = grid[s][k]
                if it is None:
                    continue
                i = s - s0
                c, o, ln = it
                toks = order[starts[c] + o: starts[c] + o + ln]
                blob[:, p + i * H:p + (i + 1) * H] = W1[c]
                blob[:, p + ns * H + i * O:p + ns * H + (i + 1) * O] = W2[c]
                xoff = p + ns * (H + O) + (int(offs[s]) - co0)
                blob[:, xoff:xoff + ln] = x[toks].T
                blob[:, s] = b1[c]
                blob[0:O, S + s] = b2[c]
        in_maps.append({"blob": blob})
    return in_maps


def _unshard(results, plan):
    S, offs = plan["S"], plan["offs"]
    order, starts, grid = plan["order"], plan["starts"], plan["grid"]
    out = np.empty((N, O), dtype=np.float32)
    for k in range(NCORES):
        outT = results[k]["outT"]
        for s in range(S):
            it = grid[s][k]
            if it is None:
                continue
            c, o, ln = it
            toks = order[starts[c] + o: starts[c] + o + ln]
            off = int(offs[s])
            out[toks] = outT[:, off:off + ln].T
    return out


def _execute(x, cat_ids, W1, b1, W2, b2, trace=False):
    x = np.asarray(x, dtype=np.float32)
    W1 = np.asarray(W1, dtype=np.float32)
    b1 = np.asarray(b1, dtype=np.float32)
    W2 = np.asarray(W2, dtype=np.float32)
    b2 = np.asarray(b2, dtype=np.float32)

    plan = _plan(cat_ids)
    nc = _build_nc(plan)
    in_maps = _shard_inputs(x, W1, b1, W2, b2, plan)
    res = run_bass_kernel_spmd(nc, in_maps, list(range(NCORES)), trace=trace)
    out = _unshard(res.results, plan)
    return out, res


def kernel(x, cat_ids, W1, b1, W2, b2):
    out, _ = _execute(x, cat_ids, W1, b1, W2, b2, trace=False)
    return out


# revision 7
# speedup vs baseline: 1.4219x; 1.2911x over previous
"""Category-specific MLP (MoE-style routing) for Trainium2, 8 NeuronCores.

Reference computation (per token n):
    h   = relu(x[n] @ W1[cat[n]] + b1[cat[n]])      x:[N,128]  W1:[100,128,128]
    out = h @ W2[cat[n]] + b2[cat[n]]               W2:[100,128,64]

Strategy (expert-parallel, MoE-style):
  * Host: sort tokens by category. Split any category with more than 512
    tokens into work items of <=512 tokens. Sort items by size (desc) and
    assign item of rank r to (core r%8, slot r//8). All cores run the same
    SPMD program with S slots; slot s has fixed column capacity caps[s] =
    size of the largest item in that slot across cores, so the instruction
    stream and shapes are identical on every core while padding stays
    minimal (~5% for the target distribution).
  * Everything is kept feature-on-partitions (transposed). Slots are packed
    into groups of <=512 total columns; one PSUM bank holds a whole group's
    layer, and one DMA loads a whole group's weights+activations from a
    single per-core fp16 blob:
        [ group0: w1 ns*128 | w2 ns*64 | xT cols | group1: ... ]
    Per slot (fp16 matmuls, fp32 PSUM accumulate):
        psum1[:, lo:lo+B] = W1_s^T @ xT_s        (PE)
        psum2[:, lo:lo+B] = W2_s^T @ h_s         (PE)
    Per group (merged PSUM evacuation on DVE, valid because b1/b2 == 0;
    a per-slot bias path is emitted instead when biases are nonzero):
        h_g   = max(psum1_g, 0)   -> fp16 SBUF
        out_g = copy(psum2_g)     -> fp32 SBUF -> DMA
    Groups are software-pipelined (layer-2 of group g emitted after layer-1
    of group g+1) so the PE stream never waits on an evacuation.
  * Host: scatter outT columns back to the original token order.

fp16 numerics: inputs are rounded to fp16 (10-bit mantissa), accumulation
is fp32 in PSUM. Measured vs the fp32 reference: resid_var ~2e-7,
absmax-relative error ~5e-4.
"""

from contextlib import ExitStack

import numpy as np

import concourse.bass as bass
import concourse.mybir as mybir
import concourse.tile as tile
from concourse import bacc
from concourse.bass_utils import run_bass_kernel_spmd

N, C, D, H, O = 8192, 100, 128, 128, 64
NCORES = 8
MAX_ITEM = 512      # PSUM bank / moving-operand limit (fp32 columns)
GROUP_COLS = 512    # column budget per group (one PSUM bank, fp32)

F16 = mybir.dt.float16
F32 = mybir.dt.float32


def _plan(cat_ids: np.ndarray, zero_bias: bool):
    """Host-side routing plan: work items -> (core, slot) assignment."""
    cat_ids = np.asarray(cat_ids).astype(np.int64)
    counts = np.bincount(cat_ids, minlength=C)
    order = np.argsort(cat_ids, kind="stable")          # token ids sorted by cat
    starts = np.zeros(C, dtype=np.int64)
    starts[1:] = np.cumsum(counts)[:-1]

    items = []                                          # (cat, start_in_cat, len)
    for c in range(C):
        cnt = int(counts[c])
        o = 0
        while o < cnt:
            ln = min(MAX_ITEM, cnt - o)
            items.append((c, o, ln))
            o += ln
    items.sort(key=lambda it: -it[2])

    S = (len(items) + NCORES - 1) // NCORES
    grid = [[None] * NCORES for _ in range(S)]          # grid[s][k] = item|None
    for r, it in enumerate(items):
        grid[r // NCORES][r % NCORES] = it
    caps = tuple(max(1, max((it[2] for it in row if it is not None), default=1))
                 for row in grid)
    offs = np.zeros(S + 1, dtype=np.int64)
    offs[1:] = np.cumsum(caps)
    T = int(offs[-1])

    # pack slots into groups of <= GROUP_COLS columns
    groups = []
    s0 = 0
    while s0 < S:
        s1 = s0 + 1
        while s1 < S and int(offs[s1 + 1] - offs[s0]) <= GROUP_COLS:
            s1 += 1
        groups.append((s0, s1))
        s0 = s1

    # fp16 blob column layout: [per group: w1 ns*H | w2 ns*O | x cols]
    gpos = []
    pos = 0
    for (s0, s1) in groups:
        ns = s1 - s0
        cols = int(offs[s1] - offs[s0])
        gpos.append(pos)
        pos += ns * (H + O) + cols
    Z = pos

    return {
        "order": order, "starts": starts, "grid": grid,
        "S": S, "caps": caps, "offs": offs, "T": T,
        "groups": groups, "gpos": gpos, "Z": Z, "zero_bias": zero_bias,
    }


_NC_CACHE: dict = {}


def _build_nc(plan):
    S, caps, T, Z = plan["S"], plan["caps"], plan["T"], plan["Z"]
    zero_bias = plan["zero_bias"]
    key = (S, caps, zero_bias)
    if key in _NC_CACHE:
        return _NC_CACHE[key]

    offs, groups, gpos = plan["offs"], plan["groups"], plan["gpos"]
    G = len(groups)

    nc = bacc.Bacc("TRN2", target_bir_lowering=False, debug=False)
    blob_d = nc.dram_tensor("blob", [128, Z], F16, kind="ExternalInput").ap()
    if not zero_bias:
        bias_d = nc.dram_tensor("bias", [128, 2 * S], F32,
                                kind="ExternalInput").ap()
    out_d = nc.dram_tensor("outT", [O, T], F32, kind="ExternalOutput").ap()

    with tile.TileContext(nc) as tc, ExitStack() as ctx:
        loads = ctx.enter_context(tc.tile_pool(name="loads", bufs=2))
        hbuf = ctx.enter_context(tc.tile_pool(name="hbuf", bufs=2))
        obuf = ctx.enter_context(tc.tile_pool(name="obuf", bufs=2))
        ps1p = ctx.enter_context(tc.tile_pool(name="ps1p", bufs=2, space="PSUM"))
        ps2p = ctx.enter_context(tc.tile_pool(name="ps2p", bufs=2, space="PSUM"))
        if not zero_bias:
            consts = ctx.enter_context(tc.tile_pool(name="consts", bufs=1))
            bias = consts.tile([128, 2 * S], F32)
            nc.sync.dma_start(out=bias, in_=bias_d)

        state = {}      # per live group: tiles needed by the layer-2 phase

        def phase1(gi):
            s0, s1 = groups[gi]
            ns = s1 - s0
            co0, co1 = int(offs[s0]), int(offs[s1])
            cols = co1 - co0
            W_g = ns * (H + O) + cols
            g_sb = loads.tile([128, W_g], F16, tag="blob", name=f"blob_{gi}")
            nc.sync.dma_start(out=g_sb, in_=blob_d[:, gpos[gi]:gpos[gi] + W_g])
            xv = g_sb[:, ns * (H + O):W_g]
            ps1 = ps1p.tile([H, cols], F32, tag="ps1", name=f"ps1_{gi}")
            for s in range(s0, s1):
                i, B = s - s0, int(caps[s])
                lo = int(offs[s]) - co0
                nc.tensor.matmul(ps1[:, lo:lo + B],
                                 lhsT=g_sb[:, i * H:(i + 1) * H],
                                 rhs=xv[:, lo:lo + B], start=True, stop=True)
            h_g = hbuf.tile([H, cols], F16, tag="h", name=f"h_{gi}")
            if zero_bias:
                nc.vector.tensor_scalar_max(h_g, ps1, 0.0)
            else:
                for s in range(s0, s1):
                    i, B = s - s0, int(caps[s])
                    lo = int(offs[s]) - co0
                    nc.vector.tensor_scalar(
                        h_g[:, lo:lo + B], ps1[:, lo:lo + B], bias[:, s:s + 1],
                        0.0, mybir.AluOpType.add, mybir.AluOpType.max)
            state[gi] = (g_sb, h_g)

        def phase2(gi):
            s0, s1 = groups[gi]
            ns = s1 - s0
            co0, co1 = int(offs[s0]), int(offs[s1])
            cols = co1 - co0
            g_sb, h_g = state.pop(gi)
            w2v = g_sb[:, ns * H:ns * (H + O)]
            ps2 = ps2p.tile([O, cols], F32, tag="ps2", name=f"ps2_{gi}")
            for s in range(s0, s1):
                i, B = s - s0, int(caps[s])
                lo = int(offs[s]) - co0
                nc.tensor.matmul(ps2[:, lo:lo + B],
                                 lhsT=g_sb[:, ns * H + i * O:ns * H + (i + 1) * O],
                                 rhs=h_g[:, lo:lo + B], start=True, stop=True)
            o_g = obuf.tile([O, cols], F32, tag="o", name=f"o_{gi}")
            if zero_bias:
                nc.vector.tensor_copy(o_g, ps2)
            else:
                for s in range(s0, s1):
                    i, B = s - s0, int(caps[s])
                    lo = int(offs[s]) - co0
                    nc.vector.tensor_scalar_add(o_g[:, lo:lo + B],
                                                ps2[:, lo:lo + B],
                                                bias[0:O, S + s:S + s + 1])
            nc.sync.dma_start(out=out_d[:, co0:co1], in_=o_g)

        # software pipeline: layer-2 of group g rides behind layer-1 of g+1
        phase1(0)
        for gi in range(1, G):
            phase1(gi)
            phase2(gi - 1)
        phase2(G - 1)

    nc.compile()
    _NC_CACHE[key] = nc
    return nc


def _shard_inputs(x, W1, b1, W2, b2, plan):
    S, offs, Z = plan["S"], plan["offs"], plan["Z"]
    order, starts, grid = plan["order"], plan["starts"], plan["grid"]
    groups, gpos = plan["groups"], plan["gpos"]

    x16 = x.astype(np.float16)
    W116 = W1.astype(np.float16)
    W216 = W2.astype(np.float16)

    in_maps = []
    for k in range(NCORES):
        blob = np.zeros((128, Z), dtype=np.float16)
        if not plan["zero_bias"]:
            biasc = np.zeros((128, 2 * S), dtype=np.float32)
        for gi, (s0, s1) in enumerate(groups):
            ns = s1 - s0
            co0 = int(offs[s0])
            p = gpos[gi]
            for s in range(s0, s1):
                it = grid[s][k]
                if it is None:
                    continue
                i = s - s0
                c, o, ln = it
                toks = order[starts[c] + o: starts[c] + o + ln]
                blob[:, p + i * H:p + (i + 1) * H] = W116[c]
                blob[:, p + ns * H + i * O:p + ns * H + (i + 1) * O] = W216[c]
                xoff = p + ns * (H + O) + (int(offs[s]) - co0)
                blob[:, xoff:xoff + ln] = x16[toks].T
                if not plan["zero_bias"]:
                    biasc[:, s] = b1[c]
                    biasc[0:O, S + s] = b2[c]
        m = {"blob": blob}
        if not plan["zero_bias"]:
            m["bias"] = biasc
        in_maps.append(m)
    return in_maps


def _unshard(results, plan):
    S, offs = plan["S"], plan["offs"]
    order, starts, grid = plan["order"], plan["starts"], plan["grid"]
    out = np.empty((N, O), dtype=np.float32)
    for k in range(NCORES):
        outT = results[k]["outT"]
        for s in range(S):
            it = grid[s][k]
            if it is None:
                continue
            c, o, ln = it
            toks = order[starts[c] + o: starts[c] + o + ln]
            off = int(offs[s])
            out[toks] = outT[:, off:off + ln].T
    return out


def _execute(x, cat_ids, W1, b1, W2, b2, trace=False):
    x = np.asarray(x, dtype=np.float32)
    W1 = np.asarray(W1, dtype=np.float32)
    b1 = np.asarray(b1, dtype=np.float32)
    W2 = np.asarray(W2, dtype=np.float32)
    b2 = np.asarray(b2, dtype=np.float32)

    zero_bias = not (b1.any() or b2.any())
    plan = _plan(cat_ids, zero_bias)
    nc = _build_nc(plan)
    in_maps = _shard_inputs(x, W1, b1, W2, b2, plan)
    res = run_bass_kernel_spmd(nc, in_maps, list(range(NCORES)), trace=trace)
    out = _unshard(res.results, plan)
    return out, res


def kernel(x, cat_ids, W1, b1, W2, b2):
    out, _ = _execute(x, cat_ids, W1, b1, W2, b2, trace=False)
    return out


# revision 11
# speedup vs baseline: 1.5361x; 1.0804x over previous
"""Category-specific MLP (MoE-style routing) for Trainium2, 8 NeuronCores.

Reference computation (per token n):
    h   = relu(x[n] @ W1[cat[n]] + b1[cat[n]])      x:[N,128]  W1:[100,128,128]
    out = h @ W2[cat[n]] + b2[cat[n]]               W2:[100,128,64]

Strategy (expert-parallel, MoE-style):
  * Host: sort tokens by category. Split any category with more than 512
    tokens into work items of <=512 tokens. Sort items by size (desc) and
    assign item of rank r to (core r%8, slot r//8). All cores run the same
    SPMD program with S slots; slot s has fixed column capacity caps[s] =
    size of the largest item in that slot across cores, so the instruction
    stream and shapes are identical on every core while padding stays
    minimal (~5% for the target distribution).
  * Everything is kept feature-on-partitions (transposed). Slots are packed
    into groups of <=512 total columns; one PSUM bank holds a whole group's
    layer, and one DMA loads a whole group's weights+activations from a
    single per-core fp16 blob:
        [ group0: w1 ns*128 | w2 ns*64 | xT cols | group1: ... ]
    Per slot (fp16 matmuls, fp32 PSUM accumulate):
        psum1[:, lo:lo+B] = W1_s^T @ xT_s        (PE)
        psum2[:, lo:lo+B] = W2_s^T @ h_s         (PE)
    Per group (merged PSUM evacuation on DVE, valid because b1/b2 == 0;
    a per-slot bias path is emitted instead when biases are nonzero):
        h_g   = max(psum1_g, 0)   -> fp16 SBUF
        out_g = copy(psum2_g)     -> fp32 SBUF -> DMA
    Groups are software-pipelined (layer-2 of group g emitted after layer-1
    of group g+1) so the PE stream never waits on an evacuation.
  * Host: scatter outT columns back to the original token order.

fp16 numerics: inputs are rounded to fp16 (10-bit mantissa), accumulation
is fp32 in PSUM. Measured vs the fp32 reference: resid_var ~2e-7,
absmax-relative error ~5e-4.
"""

from contextlib import ExitStack

import numpy as np

import concourse.bass as bass
import concourse.mybir as mybir
import concourse.tile as tile
from concourse import bacc
from concourse.bass_utils import run_bass_kernel_spmd

N, C, D, H, O = 8192, 100, 128, 128, 64
NCORES = 8
MAX_ITEM = 512      # PSUM bank / moving-operand limit (fp32 columns)
GROUP_COLS = 512    # column budget per group (one PSUM bank, fp32)

F16 = mybir.dt.float16
F32 = mybir.dt.float32


def _plan(cat_ids: np.ndarray, zero_bias: bool):
    """Host-side routing plan: work items -> (core, slot) assignment."""
    cat_ids = np.asarray(cat_ids).astype(np.int64)
    counts = np.bincount(cat_ids, minlength=C)
    order = np.argsort(cat_ids, kind="stable")          # token ids sorted by cat
    starts = np.zeros(C, dtype=np.int64)
    starts[1:] = np.cumsum(counts)[:-1]

    items = []                                          # (cat, start_in_cat, len)
    for c in range(C):
        cnt = int(counts[c])
        o = 0
        while o < cnt:
            ln = min(MAX_ITEM, cnt - o)
            items.append((c, o, ln))
            o += ln
    items.sort(key=lambda it: -it[2])

    S = (len(items) + NCORES - 1) // NCORES
    grid = [[None] * NCORES for _ in range(S)]          # grid[s][k] = item|None
    for r, it in enumerate(items):
        grid[r // NCORES][r % NCORES] = it
    caps = tuple(max(1, max((it[2] for it in row if it is not None), default=1))
                 for row in grid)
    offs = np.zeros(S + 1, dtype=np.int64)
    offs[1:] = np.cumsum(caps)
    T = int(offs[-1])

    # pack slots into groups of <= GROUP_COLS columns; keep the first group
    # tiny so its DMA lands early and the PE starts ~2us sooner
    groups = []
    s0 = 0
    while s0 < S:
        budget = 128 if s0 == 0 else GROUP_COLS
        s1 = s0 + 1
        while s1 < S and int(offs[s1 + 1] - offs[s0]) <= budget:
            s1 += 1
        groups.append((s0, s1))
        s0 = s1

    # fp16 blob column layout: [per group: w1 ns*H | w2 ns*O | x cols]
    gpos = []
    pos = 0
    for (s0, s1) in groups:
        ns = s1 - s0
        cols = int(offs[s1] - offs[s0])
        gpos.append(pos)
        pos += ns * (H + O) + cols
    Z = pos

    return {
        "order": order, "starts": starts, "grid": grid,
        "S": S, "caps": caps, "offs": offs, "T": T,
        "groups": groups, "gpos": gpos, "Z": Z, "zero_bias": zero_bias,
    }


_NC_CACHE: dict = {}


def _build_nc(plan):
    S, caps, T, Z = plan["S"], plan["caps"], plan["T"], plan["Z"]
    zero_bias = plan["zero_bias"]
    key = (S, caps, zero_bias)
    if key in _NC_CACHE:
        return _NC_CACHE[key]

    offs, groups, gpos = plan["offs"], plan["groups"], plan["gpos"]
    G = len(groups)

    nc = bacc.Bacc("TRN2", target_bir_lowering=False, debug=False,
                   enable_partition_id=False)
    blob_d = nc.dram_tensor("blob", [128, Z], F16, kind="ExternalInput").ap()
    if not zero_bias:
        bias_d = nc.dram_tensor("bias", [128, 2 * S], F32,
                                kind="ExternalInput").ap()
    out_d = nc.dram_tensor("outT", [O, T], F16, kind="ExternalOutput").ap()

    with tile.TileContext(nc) as tc, ExitStack() as ctx:
        loads = ctx.enter_context(tc.tile_pool(name="loads", bufs=max(2, G)))
        hbuf = ctx.enter_context(tc.tile_pool(name="hbuf", bufs=3))
        obuf = ctx.enter_context(tc.tile_pool(name="obuf", bufs=3))
        ps1p = ctx.enter_context(tc.tile_pool(name="ps1p", bufs=2, space="PSUM"))
        ps2p = ctx.enter_context(tc.tile_pool(name="ps2p", bufs=2, space="PSUM"))
        if not zero_bias:
            consts = ctx.enter_context(tc.tile_pool(name="consts", bufs=1))
            bias = consts.tile([128, 2 * S], F32)
            nc.sync.dma_start(out=bias, in_=bias_d)

        state = {}      # per live group: tiles needed by the layer-2 phase

        def phase1(gi):
            s0, s1 = groups[gi]
            ns = s1 - s0
            co0, co1 = int(offs[s0]), int(offs[s1])
            cols = co1 - co0
            W_g = ns * (H + O) + cols
            g_sb = loads.tile([128, W_g], F16, tag="blob", name=f"blob_{gi}")
            nc.sync.dma_start(out=g_sb, in_=blob_d[:, gpos[gi]:gpos[gi] + W_g])
            xv = g_sb[:, ns * (H + O):W_g]
            ps1 = ps1p.tile([H, cols], F32, tag="ps1", name=f"ps1_{gi}")
            for s in range(s0, s1):
                i, B = s - s0, int(caps[s])
                lo = int(offs[s]) - co0
                nc.tensor.matmul(ps1[:, lo:lo + B],
                                 lhsT=g_sb[:, i * H:(i + 1) * H],
                                 rhs=xv[:, lo:lo + B], start=True, stop=True)
            h_g = hbuf.tile([H, cols], F16, tag="h", name=f"h_{gi}")
            if zero_bias:
                nc.vector.tensor_scalar_max(h_g, ps1, 0.0)
            else:
                for s in range(s0, s1):
                    i, B = s - s0, int(caps[s])
                    lo = int(offs[s]) - co0
                    nc.vector.tensor_scalar(
                        h_g[:, lo:lo + B], ps1[:, lo:lo + B], bias[:, s:s + 1],
                        0.0, mybir.AluOpType.add, mybir.AluOpType.max)
            state[gi] = (g_sb, h_g)

        def phase2(gi):
            s0, s1 = groups[gi]
            ns = s1 - s0
            co0, co1 = int(offs[s0]), int(offs[s1])
            cols = co1 - co0
            g_sb, h_g = state.pop(gi)
            w2v = g_sb[:, ns * H:ns * (H + O)]
            ps2 = ps2p.tile([O, cols], F32, tag="ps2", name=f"ps2_{gi}")
            for s in range(s0, s1):
                i, B = s - s0, int(caps[s])
                lo = int(offs[s]) - co0
                nc.tensor.matmul(ps2[:, lo:lo + B],
                                 lhsT=g_sb[:, ns * H + i * O:ns * H + (i + 1) * O],
                                 rhs=h_g[:, lo:lo + B], start=True, stop=True)
            o_g = obuf.tile([O, cols], F16, tag="o", name=f"o_{gi}")
            if zero_bias:
                nc.vector.tensor_copy(o_g, ps2)
            else:
                for s in range(s0, s1):
                    i, B = s - s0, int(caps[s])
                    lo = int(offs[s]) - co0
                    nc.vector.tensor_scalar_add(o_g[:, lo:lo + B],
                                                ps2[:, lo:lo + B],
                                                bias[0:O, S + s:S + s + 1])
            nc.scalar.dma_start(out=out_d[:, co0:co1], in_=o_g)

        # software pipeline: layer-2 of group g rides behind layer-1 of g+1
        phase1(0)
        for gi in range(1, G):
            phase1(gi)
            phase2(gi - 1)
        phase2(G - 1)

    nc.compile()
    _NC_CACHE[key] = nc
    return nc


def _shard_inputs(x, W1, b1, W2, b2, plan):
    S, offs, Z = plan["S"], plan["offs"], plan["Z"]
    order, starts, grid = plan["order"], plan["starts"], plan["grid"]
    groups, gpos = plan["groups"], plan["gpos"]

    x16 = x.astype(np.float16)
    W116 = W1.astype(np.float16)
    W216 = W2.astype(np.float16)

    in_maps = []
    for k in range(NCORES):
        blob = np.zeros((128, Z), dtype=np.float16)
        if not plan["zero_bias"]:
            biasc = np.zeros((128, 2 * S), dtype=np.float32)
        for gi, (s0, s1) in enumerate(groups):
            ns = s1 - s0
            co0 = int(offs[s0])
            p = gpos[gi]
            for s in range(s0, s1):
                it = grid[s][k]
                if it is None:
                    continue
                i = s - s0
                c, o, ln = it
                toks = order[starts[c] + o: starts[c] + o + ln]
                blob[:, p + i * H:p + (i + 1) * H] = W116[c]
                blob[:, p + ns * H + i * O:p + ns * H + (i + 1) * O] = W216[c]
                xoff = p + ns * (H + O) + (int(offs[s]) - co0)
                blob[:, xoff:xoff + ln] = x16[toks].T
                if not plan["zero_bias"]:
                    biasc[:, s] = b1[c]
                    biasc[0:O, S + s] = b2[c]
        m = {"blob": blob}
        if not plan["zero_bias"]:
            m["bias"] = biasc
        in_maps.append(m)
    return in_maps


def _unshard(results, plan):
    S, offs = plan["S"], plan["offs"]
    order, starts, grid = plan["order"], plan["starts"], plan["grid"]
    out = np.empty((N, O), dtype=np.float32)
    for k in range(NCORES):
        outT = results[k]["outT"].astype(np.float32)
        for s in range(S):
            it = grid[s][k]
            if it is None:
                continue
            c, o, ln = it
            toks = order[starts[c] + o: starts[c] + o + ln]
            off = int(offs[s])
            out[toks] = outT[:, off:off + ln].T
    return out


def _execute(x, cat_ids, W1, b1, W2, b2, trace=False):
    x = np.asarray(x, dtype=np.float32)
    W1 = np.asarray(W1, dtype=np.float32)
    b1 = np.asarray(b1, dtype=np.float32)
    W2 = np.asarray(W2, dtype=np.float32)
    b2 = np.asarray(b2, dtype=np.float32)

    zero_bias = not (b1.any() or b2.any())
    plan = _plan(cat_ids, zero_bias)
    nc = _build_nc(plan)
    in_maps = _shard_inputs(x, W1, b1, W2, b2, plan)
    res = run_bass_kernel_spmd(nc, in_maps, list(range(NCORES)), trace=trace)
    out = _unshard(res.results, plan)
    return out, res


def kernel(x, cat_ids, W1, b1, W2, b2):
    out, _ = _execute(x, cat_ids, W1, b1, W2, b2, trace=False)
    return out
